# revision 12
# baseline (speedup 1.0000x reference)
"""Peephole-LSTM Trainium2 kernel (per-core program, SPMD over 8 cores).

Each core owns NB=16 batch rows and runs the full T-step recurrence locally
(weights replicated). v1: software-pipelined 2-wave loop.

Layout (per core):
  Wave w in {0,1} covers h-span [512w, 512w+512); strip s in {0..3} covers
  h-sub Hws = [512w+128s, 512w+128s+128).
  PSUM wave tile ps_w [128, 896], partition 32s+r (r<16) = (batch row r, Hws):
    bank0: [0:128) peep_i | [128:256) peep_f   ([256:512) unused)
    bank1: [512:640) pre_g | [640:768) pre_i | [768:896) pre_f
  (bank-separated so each bank has exactly one accumulation-start region)
  Mains: stationary = cT chunk kc (c[:,128kc:128kc+128]^T as [128,16] bf16),
  moving = packed weights; 4-way PE column tiling (strip s at col 32s).
  chunk kc lives in cT_{w=kc//4}[:, 16*(kc%4):...], produced per wave by one
  PE transpose of c_nxt_w [128,128] + one strided DVE copy.
  xw (x@W + bias, gates g,i,f) is precomputed on-device into DRAM and injected
  into PSUM via id16 matmuls as accumulation pass 0 (prepended at step start
  to defer the first state-dependent pass); tanh(peep) injected at the end.
  Chunk order within each wave's mains: wave-0 state chunks (0-3) first,
  wave-1 chunks (4-7) last, so step t+1 can start before cT_1(t) lands.
  o/h computed once per row at t=lens[b]-1 after the loop via indirect-DMA
  gathers from the c history in DRAM.
"""

import numpy as np
import ml_dtypes

import concourse.bass as bass
import concourse.bacc as bacc
import concourse.mybir as mybir
import concourse.tile as tile
from concourse.bass import ds

F32 = mybir.dt.float32
BF16 = mybir.dt.bfloat16
I32 = mybir.dt.int32
AF = mybir.ActivationFunctionType
BF = ml_dtypes.bfloat16

B, T_FULL, I_DIM, H = 128, 1024, 512, 1024
NB = 16              # batch rows per core
NS = 4               # strips (h-blocks per wave)
NW = 2               # waves
HB = 128             # h per strip per wave
KC = H // 128        # 8 k-chunks
IC = I_DIM // 128    # 4 input chunks
UPW = NS * NW * 640  # 5120 packed loop-weight cols per k-chunk
XWW = NW * 384       # 768 xw cols per step-row


# ----------------------------------------------------------------- host packing

def pack_weights(U, P, W, P_o, bias):
    # loop weights: up[kc][128][s*1280 + w*640 + blk*128 + j]
    # blk order: pi pf gg ii ff ; h_global = 512w + 128s + j
    up = np.empty((H, NS, NW, 5, HB), np.float32)
    for s in range(NS):
        for w in range(NW):
            h0 = 512 * w + 128 * s
            up[:, s, w, 0] = P[:, h0:h0 + HB]            # peep_i
            up[:, s, w, 1] = P[:, H + h0:H + h0 + HB]    # peep_f
            up[:, s, w, 2] = U[:, 2 * H + h0:2 * H + h0 + HB]  # g
            up[:, s, w, 3] = U[:, 0 * H + h0:0 * H + h0 + HB]  # i
            up[:, s, w, 4] = U[:, 1 * H + h0:1 * H + h0 + HB]  # f
    up_pack = np.ascontiguousarray(
        up.reshape(KC, 128, UPW)).astype(BF)

    # precompute weights: w_pack[ic][128][s*768 + w*384 + gate*128 + j]
    # gate order g i f
    wre = np.empty((I_DIM, NS, NW, 3, HB), np.float32)
    bre = np.empty((NS, NW, 3, HB), np.float32)
    for s in range(NS):
        for w in range(NW):
            h0 = 512 * w + 128 * s
            wre[:, s, w, 0] = W[:, 2 * H + h0:2 * H + h0 + HB]
            wre[:, s, w, 1] = W[:, 0 * H + h0:0 * H + h0 + HB]
            wre[:, s, w, 2] = W[:, 1 * H + h0:1 * H + h0 + HB]
            bre[s, w, 0] = bias[2 * H + h0:2 * H + h0 + HB]
            bre[s, w, 1] = bias[0 * H + h0:0 * H + h0 + HB]
            bre[s, w, 2] = bias[1 * H + h0:1 * H + h0 + HB]
    w_pack = np.ascontiguousarray(wre.reshape(IC, 128, NS * 768)).astype(BF)
    bias_bc = np.ascontiguousarray(
        np.broadcast_to(bre.reshape(1, NS * 768), (128, NS * 768))).astype(BF)

    uo_pack = np.ascontiguousarray(U[:, 3 * H:].reshape(KC, 128, H)).astype(BF)
    po_pack = np.ascontiguousarray(P_o.reshape(KC, 128, H)).astype(BF)
    wo_pack = np.ascontiguousarray(W[:, 3 * H:].reshape(IC, 128, H)).astype(BF)
    biaso_pack = np.ascontiguousarray(bias[3 * H:].reshape(1, H)).astype(BF)
    return dict(up=up_pack, w=w_pack, bias_bc=bias_bc, uo=uo_pack,
                po=po_pack, wo=wo_pack, biaso=biaso_pack)


def pack_consts():
    id16 = np.zeros((128, 16), dtype=BF)
    id16f = np.zeros((128, 16), dtype=np.float32)
    for s in range(NS):
        for j in range(16):
            id16[32 * s + j, j] = 1.0
            id16f[32 * s + j, j] = 1.0
    ones1 = np.ones((1, 128), dtype=BF)
    id128 = np.eye(128, dtype=np.float32)
    return dict(id16=id16, id16f=id16f, ones1=ones1, id128=id128)


def pack_core_inputs(x_core, lens_core, t_steps):
    t = t_steps
    x_tb = np.ascontiguousarray(
        x_core[:, :t, :].transpose(1, 0, 2)).reshape(t * NB, I_DIM)
    xT = np.ascontiguousarray(x_tb.T).reshape(IC, 128, t * NB).astype(BF)
    L = np.minimum(lens_core.astype(np.int64), t)
    x_f = np.ascontiguousarray(x_core[np.arange(NB), L - 1, :]).astype(BF)
    gidx = np.zeros((16, 8), np.int32)
    for r in range(NB):
        for s in range(NS):
            gidx[r, s] = int(L[r]) * 128 + 32 * s + r
            gidx[r, 4 + s] = (int(L[r]) - 1) * 128 + 32 * s + r
    return dict(xT=xT, x_f=x_f, gidx=gidx)


# ----------------------------------------------------------------- the program

def _phase_precompute(nc, tc, t, xT_in, w_in, bias_bc_in, xw_hist):
    nmt = (t * NB) // 128
    with (
        tc.tile_pool(name="wre", bufs=1) as wrep,
        tc.tile_pool(name="xt", bufs=3) as xtp,
        tc.tile_pool(name="xwsb", bufs=3) as xwsbp,
        tc.tile_pool(name="biasbc", bufs=1) as biasp,
        tc.tile_pool(name="pspre", bufs=2, space="PSUM") as psprep,  # 2x3 banks
    ):
        w_sb = wrep.tile([128, IC * 3 * H], BF16)
        for c in range(IC):
            nc.sync.dma_start(w_sb[:, c * 3 * H: (c + 1) * 3 * H], w_in[c, :, :])
        bias_sb = biasp.tile([128, 3 * H], BF16)
        nc.sync.dma_start(bias_sb[:], bias_bc_in[:])

        for m in range(nmt):
            xt_t = xtp.tile([128, IC * 128], BF16)
            for c in range(IC):
                nc.sync.dma_start(
                    xt_t[:, c * 128: (c + 1) * 128],
                    xT_in[c, :, ds(m * 128, 128)],
                )
            xw_sb = xwsbp.tile([128, 3 * H], BF16)
            for half in range(2):
                ps = psprep.tile([128, 1536], F32, tag="pp", name=f"pp{half}")
                for run in range(3):
                    cs = slice(512 * run, 512 * run + 512)
                    w0 = 1536 * half + 512 * run
                    for c in range(IC):
                        nc.tensor.matmul(
                            ps[:, cs],
                            xt_t[:, c * 128: (c + 1) * 128],
                            w_sb[:, c * 3 * H + w0: c * 3 * H + w0 + 512],
                            start=(c == 0), stop=(c == IC - 1))
                # bias add fused into the PSUM evacuation
                nc.vector.tensor_add(
                    xw_sb[:, 1536 * half: 1536 * half + 1536], ps[:],
                    bias_sb[:, 1536 * half: 1536 * half + 1536])
            # scatter: sbuf [(tau r), (s c)] -> dram rows (m*8+tau)*128+32s+r
            for s in range(NS):
                nc.sync.dma_start(
                    xw_hist[ds(m * 8 * 128, 8 * 128), :].rearrange(
                        "(tau q) j -> tau q j", q=128)[:, 32 * s: 32 * s + NB, :],
                    xw_sb[:, 768 * s: 768 * (s + 1)],
                )


def _phase_loop(nc, tc, cst, t, up_in, xw_hist, c_hist):
    id16 = cst["id16"]
    id128 = cst["id128"]
    with (
        tc.tile_pool(name="upw", bufs=1) as upp,
        tc.tile_pool(name="xwblk", bufs=4) as xwblkp,
        tc.tile_pool(name="state", bufs=1) as statep,
        tc.tile_pool(name="eltw", bufs=3) as eltp,
        tc.tile_pool(name="psmain", bufs=3, space="PSUM") as psmp,  # 6 banks
        tc.tile_pool(name="pstr", bufs=2, space="PSUM") as pstrp,   # 2 banks
    ):
        up_sb = []
        for k in range(KC):
            u = upp.tile([128, UPW], BF16, tag=f"up{k}", name=f"up{k}")
            nc.sync.dma_start(u[:], up_in[k, :, :])
            up_sb.append(u)

        # state: per wave, ping-pong (step parity)
        c_t = [[statep.tile([128, 128], F32, tag=f"c{w}{p}", name=f"c{w}{p}")
                for p in range(2)] for w in range(NW)]
        cT_t = [[statep.tile([128, 64], BF16, tag=f"cT{w}{p}", name=f"cT{w}{p}")
                 for p in range(2)] for w in range(NW)]
        for w in range(NW):
            nc.vector.memset(c_t[w][0][:], 0.0)
            nc.vector.memset(cT_t[w][0][:], 0.0)
        czero = statep.tile([128, 256], F32, tag="czero", name="czero")
        nc.vector.memset(czero[:], 0.0)
        nc.sync.dma_start(c_hist[ds(0, 128), :], czero[:])

        for step in range(t):
            pp = step % 2
            np_ = (step + 1) % 2

            xws = xwblkp.tile([128, XWW], BF16, tag="xws", name="xws")
            nc.sync.dma_start(xws[:], xw_hist[ds(step * 128, 128), :])

            # single tag: both waves rotate through one 3-slot ring (6 banks)
            ps_w = [psmp.tile([128, 896], F32, tag="ps", name=f"ps{w}")
                    for w in range(NW)]

            # --- xw injection = bank-1 accumulation start, both waves up
            # front (defers the first state-dependent PE pass).
            for w in range(NW):
                for s in range(NS):
                    sl = slice(32 * s, 32 * s + NB)
                    tpos = (32 * s, 32 * s)
                    nc.tensor.matmul(
                        ps_w[w][sl, 512:896], id16[sl, :],
                        xws[sl, 384 * w: 384 * w + 384],
                        start=True, stop=False, tile_position=tpos,
                        skip_group_check=True)

            for w in range(NW):
                ps = ps_w[w]
                # --- mains: 8 chunk passes, wave-0 state chunks first
                for idx, kc in enumerate(range(KC)):
                    src = cT_t[kc // 4][pp]
                    lhsT = src[:, 16 * (kc % 4): 16 * (kc % 4) + 16]
                    first, last = idx == 0, idx == KC - 1
                    for s in range(NS):
                        sl = slice(32 * s, 32 * s + NB)
                        tpos = (0, 32 * s)
                        mv0 = 1280 * s + 640 * w
                        nc.tensor.matmul(
                            ps[sl, 0:256], lhsT, up_sb[kc][:, mv0:mv0 + 256],
                            start=first, stop=last, tile_position=tpos,
                            skip_group_check=True)
                        if last:
                            nc.tensor.matmul(
                                ps[sl, 512:640], lhsT,
                                up_sb[kc][:, mv0 + 256:mv0 + 384],
                                start=False, stop=True, tile_position=tpos,
                                skip_group_check=True)
                            nc.tensor.matmul(
                                ps[sl, 640:896], lhsT,
                                up_sb[kc][:, mv0 + 384:mv0 + 640],
                                start=False, stop=False, tile_position=tpos,
                                skip_group_check=True)
                        else:
                            nc.tensor.matmul(
                                ps[sl, 512:896], lhsT,
                                up_sb[kc][:, mv0 + 256:mv0 + 640],
                                start=False, stop=False, tile_position=tpos,
                                skip_group_check=True)

                # --- tail
                tp = eltp.tile([128, 256], BF16, tag=f"tp{w}", name=f"tp{w}")
                nc.scalar.activation(tp[:], ps[:, 0:256], AF.Tanh)
                # inject tanh(peep_i) into pre_i, tanh(peep_f) into pre_f
                for s in range(NS):
                    sl = slice(32 * s, 32 * s + NB)
                    tpos = (32 * s, 32 * s)
                    nc.tensor.matmul(
                        ps[sl, 640:896], id16[sl, :], tp[sl, 0:256],
                        start=False, stop=True, tile_position=tpos,
                        skip_group_check=True)

                tg = eltp.tile([128, 128], F32, tag=f"tg{w}", name=f"tg{w}")
                nc.scalar.activation(tg[:], ps[:, 512:640], AF.Tanh)
                sif = eltp.tile([128, 256], F32, tag=f"sif{w}", name=f"sif{w}")
                nc.scalar.activation(sif[:], ps[:, 640:896], AF.Sigmoid)

                t1 = eltp.tile([128, 128], F32, tag=f"t1{w}", name=f"t1{w}")
                t2 = eltp.tile([128, 128], F32, tag=f"t2{w}", name=f"t2{w}")
                nc.vector.tensor_mul(t1[:], sif[:, 0:128], tg[:])
                nc.vector.tensor_mul(t2[:], sif[:, 128:256], c_t[w][pp][:])
                c_nxt = c_t[w][np_]
                nc.vector.tensor_add(c_nxt[:], t1[:], t2[:])

                pst = pstrp.tile([128, 128], F32, tag="pst", name=f"pst{w}")
                nc.tensor.transpose(pst[:, :], c_nxt[:], id128[:, :])
                nc.vector.tensor_copy(
                    cT_t[w][np_][:].rearrange("p (s q) -> p s q", s=4),
                    pst[:].rearrange("p (s q) -> p s q", s=4)[:, :, 0:16],
                )
                # store in baseline layout: row 32g+r = (row r, h slice
                # [256g, 256g+256)); g = 2w + s//2, col half = s%2
                for s in range(NS):
                    g = 2 * w + s // 2
                    beta = s % 2
                    nc.sync.dma_start(
                        c_hist[ds((step + 1) * 128 + 32 * g, 32),
                               128 * beta: 128 * beta + 128],
                        c_nxt[32 * s: 32 * s + 32, :])


def _phase_finalize(nc, tc, cst, uo_in, po_in, wo_in, biaso_in,
                    gidx_in, xf_in, c_hist, h_out):
    id16, id16f, ones1 = cst["id16"], cst["id16f"], cst["ones1"]
    with (
        tc.tile_pool(name="finw", bufs=1) as finwp,
        tc.tile_pool(name="fin", bufs=1) as finp,
        tc.tile_pool(name="psfin", bufs=1, space="PSUM") as psfp,
        tc.tile_pool(name="psfin2", bufs=1, space="PSUM") as psf2p,
        tc.tile_pool(name="pstf", bufs=1, space="PSUM") as pstfp,
    ):
        uo_sb = finwp.tile([128, KC * H], BF16, tag="uo", name="uo")
        po_sb = finwp.tile([128, KC * H], BF16, tag="po", name="po")
        wo_sb = finwp.tile([128, IC * H], BF16, tag="wo", name="wo")
        for k in range(KC):
            nc.sync.dma_start(uo_sb[:, k * H: (k + 1) * H], uo_in[k, :, :])
            nc.sync.dma_start(po_sb[:, k * H: (k + 1) * H], po_in[k, :, :])
        for c in range(IC):
            nc.sync.dma_start(wo_sb[:, c * H: (c + 1) * H], wo_in[c, :, :])
        bo_sb = finp.tile([1, H], BF16, tag="bo", name="bo")
        nc.sync.dma_start(bo_sb[:], biaso_in[:])
        gidx = finp.tile([16, 8], I32, tag="gidx", name="gidx")
        nc.sync.dma_start(gidx[:], gidx_in[:])
        xf_sb = finp.tile([NB, I_DIM], BF16, tag="xf", name="xf")
        nc.sync.dma_start(xf_sb[:], xf_in[:])

        # gather c rows: c_hist row L*128+32g+r holds (row r, h[256g:256g+256))
        HG = 256
        cout_b = finp.tile([NB, H], F32, tag="cout", name="cout")
        cin_b = finp.tile([NB, H], F32, tag="cin", name="cin")
        for g in range(NS):
            nc.gpsimd.indirect_dma_start(
                out=cout_b[:, HG * g: HG * (g + 1)], out_offset=None,
                in_=c_hist[:],
                in_offset=bass.IndirectOffsetOnAxis(ap=gidx[:, g: g + 1], axis=0),
            )
            nc.gpsimd.indirect_dma_start(
                out=cin_b[:, HG * g: HG * (g + 1)], out_offset=None,
                in_=c_hist[:],
                in_offset=bass.IndirectOffsetOnAxis(ap=gidx[:, 4 + g: 5 + g], axis=0),
            )

        def transpose_to_bf16(src_b, nm, nchunk, ident, psum_dtype):
            dst = finp.tile([128, 16 * nchunk], BF16, tag=nm, name=nm)
            pstf = pstfp.tile([128, 128], psum_dtype, tag="pstf" + nm,
                              name="pstf" + nm)
            for k in range(nchunk):
                nc.tensor.transpose(
                    pstf[:, 16 * k: 16 * k + 16],
                    src_b[0:NB, 128 * k: 128 * k + 128],
                    ident[0:16, 0:16],
                )
            nc.vector.tensor_copy(dst[:], pstf[:, 0: 16 * nchunk])
            return dst

        xtT = transpose_to_bf16(xf_sb, "xtT", IC, id16, BF16)
        cinT = transpose_to_bf16(cin_b, "cinT", KC, id16f, F32)
        coutT = transpose_to_bf16(cout_b, "coutT", KC, id16f, F32)

        ps_o = psfp.tile([NB, H], F32)
        ps_po = psf2p.tile([NB, H], F32)
        for half in range(2):
            cs = slice(512 * half, 512 * half + 512)
            nc.tensor.matmul(ps_o[:, cs], ones1[0:1, 0:NB], bo_sb[0:1, cs],
                             start=True, stop=False)
            for c in range(IC):
                nc.tensor.matmul(
                    ps_o[:, cs], xtT[:, 16 * c: 16 * c + 16],
                    wo_sb[:, c * H + 512 * half: c * H + 512 * half + 512],
                    start=False, stop=False)
            for k in range(KC):
                nc.tensor.matmul(
                    ps_o[:, cs], cinT[:, 16 * k: 16 * k + 16],
                    uo_sb[:, k * H + 512 * half: k * H + 512 * half + 512],
                    start=False, stop=False)
                nc.tensor.matmul(
                    ps_po[:, cs], coutT[:, 16 * k: 16 * k + 16],
                    po_sb[:, k * H + 512 * half: k * H + 512 * half + 512],
                    start=(k == 0), stop=(k == KC - 1))
        tpo = finp.tile([NB, H], BF16, tag="tpo", name="tpo")
        nc.scalar.activation(tpo[:], ps_po[:], AF.Tanh)
        for half in range(2):
            cs = slice(512 * half, 512 * half + 512)
            nc.tensor.matmul(ps_o[:, cs], id16[0:NB, :], tpo[:, cs],
                             start=False, stop=True, skip_group_check=True)
        o_sb = finp.tile([NB, H], F32, tag="osb", name="osb")
        nc.scalar.activation(o_sb[:], ps_o[:], AF.Sigmoid)
        tanc = finp.tile([NB, H], F32, tag="tanc", name="tanc")
        nc.scalar.activation(tanc[:], cout_b[:], AF.Tanh)
        h_sb = finp.tile([NB, H], F32, tag="hsb", name="hsb")
        nc.vector.tensor_mul(h_sb[:], o_sb[:], tanc[:])
        nc.sync.dma_start(h_out[:], h_sb[:])


def build_program(t_steps=T_FULL, parts=("pre", "loop", "fin"),
                  c_hist_out=False):
    t = t_steps
    assert (t * NB) % 128 == 0

    nc = bacc.Bacc(None, target_bir_lowering=False, debug=False)
    dp = nc.declare_dram_parameter
    up_in = dp("up", [KC, 128, UPW], BF16, isOutput=False)
    w_in = dp("w", [IC, 128, NS * 768], BF16, isOutput=False)
    bias_bc_in = dp("bias_bc", [128, NS * 768], BF16, isOutput=False)
    uo_in = dp("uo", [KC, 128, H], BF16, isOutput=False)
    po_in = dp("po", [KC, 128, H], BF16, isOutput=False)
    wo_in = dp("wo", [IC, 128, H], BF16, isOutput=False)
    biaso_in = dp("biaso", [1, H], BF16, isOutput=False)
    xT_in = dp("xT", [IC, 128, t * NB], BF16, isOutput=False)
    xf_in = dp("x_f", [NB, I_DIM], BF16, isOutput=False)
    gidx_in = dp("gidx", [16, 8], I32, isOutput=False)
    id16_in = dp("id16", [128, 16], BF16, isOutput=False)
    id16f_in = dp("id16f", [128, 16], F32, isOutput=False)
    id128_in = dp("id128", [128, 128], F32, isOutput=False)
    ones1_in = dp("ones1", [1, 128], BF16, isOutput=False)
    h_out = dp("h_out", [NB, H], F32, isOutput=True)

    xw_hist = nc.dram_tensor("xw_hist", [t * 128, XWW], BF16)
    if c_hist_out:
        c_hist = dp("c_hist", [(t + 1) * 128, 256], F32, isOutput=True)
    else:
        c_hist = nc.dram_tensor("c_hist", [(t + 1) * 128, 256], F32)

    with tile.TileContext(nc) as tc:
        with tc.tile_pool(name="const", bufs=1) as constp:
            id16 = constp.tile([128, 16], BF16)
            nc.sync.dma_start(id16[:], id16_in[:])
            id16f = constp.tile([128, 16], F32)
            nc.sync.dma_start(id16f[:], id16f_in[:])
            ones1 = constp.tile([1, 128], BF16)
            nc.sync.dma_start(ones1[:], ones1_in[:])
            id128 = constp.tile([128, 128], F32)
            nc.sync.dma_start(id128[:], id128_in[:])
            cst = dict(id16=id16, id16f=id16f, ones1=ones1, id128=id128)

            if "pre" in parts:
                _phase_precompute(nc, tc, t, xT_in, w_in, bias_bc_in, xw_hist)
            if "loop" in parts:
                _phase_loop(nc, tc, cst, t, up_in, xw_hist, c_hist)
            if "fin" in parts:
                _phase_finalize(nc, tc, cst, uo_in, po_in, wo_in, biaso_in,
                                gidx_in, xf_in, c_hist, h_out)
            else:
                with tc.tile_pool(name="dummy", bufs=1) as dummyp:
                    hz = dummyp.tile([NB, H], F32)
                    nc.vector.memset(hz[:], 0.0)
                    nc.sync.dma_start(h_out[:], hz[:])

    nc.compile()
    return nc


# ------------------------------------------------------- full host-side kernel

def make_in_maps(inputs, t_steps):
    x = np.asarray(inputs["x"], np.float32)
    lens = np.asarray(inputs["lens"]).astype(np.int64)
    wp = pack_weights(np.asarray(inputs["U"], np.float32),
                      np.asarray(inputs["P"], np.float32),
                      np.asarray(inputs["W"], np.float32),
                      np.asarray(inputs["P_o"], np.float32),
                      np.asarray(inputs["bias"], np.float32))
    cp = pack_consts()
    shared = {**wp, **cp}
    in_maps = []
    for core in range(8):
        sl = slice(core * NB, (core + 1) * NB)
        ci = pack_core_inputs(x[sl], lens[sl], t_steps)
        in_maps.append({**shared, **ci})
    return in_maps


def run(inputs, t_steps=T_FULL, trace=False, parts=("pre", "loop", "fin"),
        c_hist_out=False, nc_cache={}):
    from concourse.bass_utils import run_bass_kernel_spmd

    in_maps = make_in_maps(inputs, t_steps)
    key = (t_steps, parts, c_hist_out)
    if key not in nc_cache:
        nc_cache[key] = build_program(t_steps, parts, c_hist_out)
    nc = nc_cache[key]

    res = run_bass_kernel_spmd(nc, in_maps, list(range(8)), trace=trace)
    h = np.concatenate([res.results[i]["h_out"] for i in range(8)], axis=0)
    return h.astype(np.float32), res


# ======================================================================
# Public entry point: full inputs in, full output out.
# ======================================================================

LAST_EXEC_NS = None


def kernel(**inputs):
    """Peephole-LSTM forward; returns h at t=lens-1 for each row: [B, H] f32."""
    global LAST_EXEC_NS
    import os
    trace = bool(os.environ.get("BASS_TRACE"))
    h, res = run(inputs, t_steps=T_FULL, trace=trace)
    if res.exec_time_ns is not None:
        LAST_EXEC_NS = res.exec_time_ns
    return h


# revision 25
# speedup vs baseline: 1.2591x; 1.2591x over previous
"""Peephole-LSTM Trainium2 kernel (per-core program, SPMD over 8 cores).

Each core owns NB=16 batch rows and runs the full T-step recurrence locally
(weights replicated). v1: software-pipelined 2-wave loop.

Layout (per core):
  Wave w in {0,1} covers h-span [512w, 512w+512); strip s in {0..3} covers
  h-sub Hws = [512w+128s, 512w+128s+128).
  PSUM wave tile ps_w [128, 896], partition 32s+r (r<16) = (batch row r, Hws):
    bank0: [0:128) peep_i | [128:256) peep_f   ([256:512) unused)
    bank1: [512:640) pre_g | [640:768) pre_i | [768:896) pre_f
  (bank-separated so each bank has exactly one accumulation-start region)
  Mains: stationary = cT chunk kc (c[:,128kc:128kc+128]^T as [128,16] bf16),
  moving = packed weights; 4-way PE column tiling (strip s at col 32s).
  chunk kc lives in cT_{w=kc//4}[:, 16*(kc%4):...], produced per wave by one
  PE transpose of c_nxt_w [128,128] + one strided DVE copy.
  xw (x@W + bias, gates g,i,f) is precomputed on-device into DRAM and injected
  into PSUM via id16 matmuls as accumulation pass 0 (prepended at step start
  to defer the first state-dependent pass); tanh(peep) injected at the end.
  Chunk order within each wave's mains: wave-0 state chunks (0-3) first,
  wave-1 chunks (4-7) last, so step t+1 can start before cT_1(t) lands.
  o/h computed once per row at t=lens[b]-1 after the loop via indirect-DMA
  gathers from the c history in DRAM.
"""

import numpy as np
import ml_dtypes

import concourse.bass as bass
import concourse.bacc as bacc
import concourse.mybir as mybir
import concourse.tile as tile
from concourse.bass import ds

F32 = mybir.dt.float32
BF16 = mybir.dt.bfloat16
I32 = mybir.dt.int32
AF = mybir.ActivationFunctionType
BF = ml_dtypes.bfloat16

B, T_FULL, I_DIM, H = 128, 1024, 512, 1024
NB = 16              # batch rows per core
NS = 4               # strips (h-blocks per wave)
NW = 2               # waves
HB = 128             # h per strip per wave
KC = H // 128        # 8 k-chunks
IC = I_DIM // 128    # 4 input chunks
UPW = NS * NW * 640  # 5120 packed loop-weight cols per k-chunk
XWW = NW * 384       # 768 xw cols per step-row


# ----------------------------------------------------------------- host packing

def pack_weights(U, P, W, P_o, bias):
    # loop weights: up[kc][128][s*1280 + w*640 + blk*128 + j]
    # blk order: pi pf gg ii ff ; h_global = 512w + 128s + j
    up = np.empty((H, NS, NW, 5, HB), np.float32)
    for s in range(NS):
        for w in range(NW):
            h0 = 512 * w + 128 * s
            up[:, s, w, 0] = P[:, h0:h0 + HB]            # peep_i
            up[:, s, w, 1] = P[:, H + h0:H + h0 + HB]    # peep_f
            up[:, s, w, 2] = U[:, 2 * H + h0:2 * H + h0 + HB]  # g
            up[:, s, w, 3] = U[:, 0 * H + h0:0 * H + h0 + HB]  # i
            up[:, s, w, 4] = U[:, 1 * H + h0:1 * H + h0 + HB]  # f
    up_pack = np.ascontiguousarray(
        up.reshape(KC, 128, UPW)).astype(BF)

    # precompute weights: w_pack[ic][128][s*768 + w*384 + gate*128 + j]
    # gate order g i f
    wre = np.empty((I_DIM, NS, NW, 3, HB), np.float32)
    bre = np.empty((NS, NW, 3, HB), np.float32)
    for s in range(NS):
        for w in range(NW):
            h0 = 512 * w + 128 * s
            wre[:, s, w, 0] = W[:, 2 * H + h0:2 * H + h0 + HB]
            wre[:, s, w, 1] = W[:, 0 * H + h0:0 * H + h0 + HB]
            wre[:, s, w, 2] = W[:, 1 * H + h0:1 * H + h0 + HB]
            bre[s, w, 0] = bias[2 * H + h0:2 * H + h0 + HB]
            bre[s, w, 1] = bias[0 * H + h0:0 * H + h0 + HB]
            bre[s, w, 2] = bias[1 * H + h0:1 * H + h0 + HB]
    w_pack = np.ascontiguousarray(wre.reshape(IC, 128, NS * 768)).astype(BF)
    bias_bc = np.ascontiguousarray(
        np.broadcast_to(bre.reshape(1, NS * 768), (128, NS * 768))).astype(BF)

    uo_pack = np.ascontiguousarray(U[:, 3 * H:].reshape(KC, 128, H)).astype(BF)
    po_pack = np.ascontiguousarray(P_o.reshape(KC, 128, H)).astype(BF)
    wo_pack = np.ascontiguousarray(W[:, 3 * H:].reshape(IC, 128, H)).astype(BF)
    biaso_pack = np.ascontiguousarray(bias[3 * H:].reshape(1, H)).astype(BF)
    return dict(up=up_pack, w=w_pack, bias_bc=bias_bc, uo=uo_pack,
                po=po_pack, wo=wo_pack, biaso=biaso_pack)


def pack_consts():
    id16 = np.zeros((128, 16), dtype=BF)
    id16f = np.zeros((128, 16), dtype=np.float32)
    for s in range(NS):
        for j in range(16):
            id16[32 * s + j, j] = 1.0
            id16f[32 * s + j, j] = 1.0
    ones1 = np.ones((1, 128), dtype=BF)
    id128 = np.eye(128, dtype=np.float32)
    id128b = np.eye(128).astype(BF)
    return dict(id16=id16, id16f=id16f, ones1=ones1, id128=id128,
                id128b=id128b)


def pack_core_inputs(x_core, lens_core, t_steps):
    t = t_steps
    x_tb = np.ascontiguousarray(
        x_core[:, :t, :].transpose(1, 0, 2)).reshape(t * NB, I_DIM)
    xT = np.ascontiguousarray(x_tb.T).reshape(IC, 128, t * NB).astype(BF)
    L = np.minimum(lens_core.astype(np.int64), t)
    x_f = np.ascontiguousarray(x_core[np.arange(NB), L - 1, :]).astype(BF)
    gidx = np.zeros((16, 8), np.int32)
    for r in range(NB):
        for s in range(NS):
            gidx[r, s] = int(L[r]) * 128 + 32 * s + r
            gidx[r, 4 + s] = (int(L[r]) - 1) * 128 + 32 * s + r
    return dict(xT=xT, x_f=x_f, gidx=gidx)


# ----------------------------------------------------------------- the program

def _phase_precompute(nc, tc, t, xT_in, w_in, bias_bc_in, xw_hist):
    nmt = (t * NB) // 128
    with (
        tc.tile_pool(name="wre", bufs=1) as wrep,
        tc.tile_pool(name="xt", bufs=3) as xtp,
        tc.tile_pool(name="xwsb", bufs=3) as xwsbp,
        tc.tile_pool(name="biasbc", bufs=1) as biasp,
        tc.tile_pool(name="pspre", bufs=2, space="PSUM") as psprep,  # 2x3 banks
    ):
        w_sb = wrep.tile([128, IC * 3 * H], BF16)
        for c in range(IC):
            nc.sync.dma_start(w_sb[:, c * 3 * H: (c + 1) * 3 * H], w_in[c, :, :])
        bias_sb = biasp.tile([128, 3 * H], BF16)
        nc.sync.dma_start(bias_sb[:], bias_bc_in[:])

        for m in range(nmt):
            xt_t = xtp.tile([128, IC * 128], BF16)
            for c in range(IC):
                nc.sync.dma_start(
                    xt_t[:, c * 128: (c + 1) * 128],
                    xT_in[c, :, ds(m * 128, 128)],
                )
            xw_sb = xwsbp.tile([128, 3 * H], BF16)
            for half in range(2):
                ps = psprep.tile([128, 1536], F32, tag="pp", name=f"pp{half}")
                for run in range(3):
                    cs = slice(512 * run, 512 * run + 512)
                    w0 = 1536 * half + 512 * run
                    for c in range(IC):
                        nc.tensor.matmul(
                            ps[:, cs],
                            xt_t[:, c * 128: (c + 1) * 128],
                            w_sb[:, c * 3 * H + w0: c * 3 * H + w0 + 512],
                            start=(c == 0), stop=(c == IC - 1))
                # bias add fused into the PSUM evacuation
                nc.vector.tensor_add(
                    xw_sb[:, 1536 * half: 1536 * half + 1536], ps[:],
                    bias_sb[:, 1536 * half: 1536 * half + 1536])
            # scatter: sbuf [(tau r), (s c)] -> dram rows (m*8+tau)*128+32s+r
            for s in range(NS):
                nc.sync.dma_start(
                    xw_hist[ds(m * 8 * 128, 8 * 128), :].rearrange(
                        "(tau q) j -> tau q j", q=128)[:, 32 * s: 32 * s + NB, :],
                    xw_sb[:, 768 * s: 768 * (s + 1)],
                )


def _phase_loop(nc, tc, cst, t, up_in, xw_hist, c_hist):
    id128 = cst["id128"]
    id128b = cst["id128b"]
    with (
        tc.tile_pool(name="upw", bufs=1) as upp,
        tc.tile_pool(name="xwblk", bufs=4) as xwblkp,
        tc.tile_pool(name="state", bufs=1) as statep,
        tc.tile_pool(name="eltw", bufs=3) as eltp,
        tc.tile_pool(name="psmain", bufs=3, space="PSUM") as psmp,  # 6 banks
        tc.tile_pool(name="pstr", bufs=2, space="PSUM") as pstrp,   # 2 banks
    ):
        up_sb = []
        for k in range(KC):
            u = upp.tile([128, UPW], BF16, tag=f"up{k}", name=f"up{k}")
            nc.sync.dma_start(u[:], up_in[k, :, :])
            up_sb.append(u)

        # state: per wave, ping-pong (step parity)
        c_t = [[statep.tile([128, 128], F32, tag=f"c{w}{p}", name=f"c{w}{p}")
                for p in range(2)] for w in range(NW)]
        # 80 cols: [0:64) = 4 chunks x 16, [64:80) = finite filler so the
        # M=32-wide stationary slices [16s:16s+32) stay in range for s=3
        cT_t = [[statep.tile([128, 80], BF16, tag=f"cT{w}{p}", name=f"cT{w}{p}")
                 for p in range(2)] for w in range(NW)]
        for w in range(NW):
            nc.vector.memset(c_t[w][0][:], 0.0)
            nc.vector.memset(cT_t[w][0][:], 0.0)
        czero = statep.tile([128, 256], F32, tag="czero", name="czero")
        nc.vector.memset(czero[:], 0.0)
        nc.sync.dma_start(c_hist[ds(0, 128), :], czero[:])

        def emit_passes(ps, w, pp, kcs):
            """mains passes for wave w over state chunks kcs."""
            for kc in kcs:
                src = cT_t[kc // 4][pp]
                # M=32 (junk cols finite) so every strip partition is written
                lhsT = src[:, 16 * (kc % 4): 16 * (kc % 4) + 32]
                first, last = kc == 0, kc == KC - 1
                for s in range(NS):
                    sl = slice(32 * s, 32 * s + 32)
                    tpos = (0, 32 * s)
                    mv0 = 1280 * s + 640 * w
                    nc.tensor.matmul(
                        ps[sl, 0:256], lhsT, up_sb[kc][:, mv0:mv0 + 256],
                        start=first, stop=last, tile_position=tpos,
                        skip_group_check=True)
                    if last:
                        nc.tensor.matmul(
                            ps[sl, 512:640], lhsT,
                            up_sb[kc][:, mv0 + 256:mv0 + 384],
                            start=False, stop=True, tile_position=tpos,
                            skip_group_check=True)
                        nc.tensor.matmul(
                            ps[sl, 640:896], lhsT,
                            up_sb[kc][:, mv0 + 384:mv0 + 640],
                            start=False, stop=False, tile_position=tpos,
                            skip_group_check=True)
                    else:
                        nc.tensor.matmul(
                            ps[sl, 512:896], lhsT,
                            up_sb[kc][:, mv0 + 256:mv0 + 640],
                            start=False, stop=False, tile_position=tpos,
                            skip_group_check=True)

        def emit_xw_inj(ps, w, xws):
            nc.tensor.matmul(ps[:, 512:896], id128b[:, :],
                             xws[:, 384 * w: 384 * w + 384],
                             start=True, stop=False, skip_group_check=True)

        def emit_tp_inj(ps, tp):
            nc.tensor.matmul(ps[:, 640:896], id128b[:, :], tp[:, 0:256],
                             start=False, stop=True, skip_group_check=True)

        def emit_acts(ps, w, pp, np_):
            """tg/sif + elementwise for wave w (after tp inject)."""
            tg = eltp.tile([128, 128], F32, tag=f"tg{w}", name=f"tg{w}")
            nc.scalar.activation(tg[:], ps[:, 512:640], AF.Tanh)
            sif = eltp.tile([128, 256], F32, tag=f"sif{w}", name=f"sif{w}")
            nc.scalar.activation(sif[:], ps[:, 640:896], AF.Sigmoid)
            t1 = eltp.tile([128, 128], F32, tag=f"t1{w}", name=f"t1{w}")
            t2 = eltp.tile([128, 128], F32, tag=f"t2{w}", name=f"t2{w}")
            nc.vector.tensor_mul(t1[:], sif[:, 0:128], tg[:])
            nc.vector.tensor_mul(t2[:], sif[:, 128:256], c_t[w][pp][:])
            c_nxt = c_t[w][np_]
            nc.vector.tensor_add(c_nxt[:], t1[:], t2[:])
            return c_nxt

        def emit_tr_copy(w, par, step_of_c):
            """transpose c(w, parity par) -> cT + c_hist store."""
            c_nxt = c_t[w][par]
            pst = pstrp.tile([128, 128], F32, tag="pst", name=f"pst{w}")
            nc.tensor.transpose(pst[:, :], c_nxt[:], id128[:, :])
            nc.vector.tensor_copy(
                cT_t[w][par][:, 0:64].rearrange("p (s q) -> p s q", s=4),
                pst[:].rearrange("p (s q) -> p s q", s=4)[:, :, 0:16],
            )
            nc.vector.tensor_copy(cT_t[w][par][:, 64:80], pst[:, 0:16])
            emit_chist(w, par, step_of_c)

        def emit_chist(w, par, step_of_c):
            c_nxt = c_t[w][par]
            for s in range(NS):
                g = 2 * w + s // 2
                beta = s % 2
                nc.sync.dma_start(
                    c_hist[ds((step_of_c + 1) * 128 + 32 * g, 32),
                           128 * beta: 128 * beta + 128],
                    c_nxt[32 * s: 32 * s + 32, :])

        def load_xws(step):
            # zero first, then DMA only the valid 16-row stripes: the junk
            # rows of xw_hist were never written and may hold NaN patterns,
            # which would poison the K=128 identity injects (0*NaN=NaN)
            xws = xwblkp.tile([128, XWW], BF16, tag="xws", name="xws")
            nc.vector.memset(xws[:], 0.0)
            for s in range(NS):
                nc.sync.dma_start(
                    xws[32 * s: 32 * s + NB, :],
                    xw_hist[ds(step * 128 + 32 * s, NB), :])
            return xws

        # software-pipelined emission: wave B(t)'s transpose is deferred
        # into step t+1's wave-A mains stream so the PE never waits on a
        # full elementwise tail.
        xws_cur = load_xws(0)
        for step in range(t):
            pp = step % 2
            np_ = (step + 1) % 2

            xws = xws_cur
            ps_w = [psmp.tile([128, 896], F32, tag="ps", name=f"ps{w}")
                    for w in range(NW)]

            emit_xw_inj(ps_w[0], 0, xws)
            emit_passes(ps_w[0], 0, pp, [0, 1, 2])
            if step > 0:
                # wave B(t-1): transpose + cT copy (c_nxt_B(t-1) = c_t[1][pp])
                emit_tr_copy(1, pp, step - 1)
            emit_passes(ps_w[0], 0, pp, [3, 4, 5, 6, 7])

            tpA = eltp.tile([128, 256], BF16, tag="tp0", name="tp0")
            nc.scalar.activation(tpA[:], ps_w[0][:, 0:256], AF.Tanh)

            emit_xw_inj(ps_w[1], 1, xws)
            emit_passes(ps_w[1], 1, pp, [0, 1, 2])
            emit_tp_inj(ps_w[0], tpA)
            emit_acts(ps_w[0], 0, pp, np_)
            emit_passes(ps_w[1], 1, pp, [3, 4, 5, 6, 7])

            # wave A(t): transpose + cT copy + store
            emit_tr_copy(0, np_, step)

            if step + 1 < t:
                xws_cur = load_xws(step + 1)

            tpB = eltp.tile([128, 256], BF16, tag="tp1", name="tp1")
            nc.scalar.activation(tpB[:], ps_w[1][:, 0:256], AF.Tanh)
            emit_tp_inj(ps_w[1], tpB)
            emit_acts(ps_w[1], 1, pp, np_)

        # epilogue: last step's wave-B c store (no transpose needed)
        emit_chist(1, t % 2, t - 1)


def _phase_finalize(nc, tc, cst, uo_in, po_in, wo_in, biaso_in,
                    gidx_in, xf_in, c_hist, h_out):
    id16, id16f, ones1 = cst["id16"], cst["id16f"], cst["ones1"]
    with (
        tc.tile_pool(name="finw", bufs=1) as finwp,
        tc.tile_pool(name="fin", bufs=1) as finp,
        tc.tile_pool(name="psfin", bufs=1, space="PSUM") as psfp,
        tc.tile_pool(name="psfin2", bufs=1, space="PSUM") as psf2p,
        tc.tile_pool(name="pstf", bufs=1, space="PSUM") as pstfp,
    ):
        uo_sb = finwp.tile([128, KC * H], BF16, tag="uo", name="uo")
        po_sb = finwp.tile([128, KC * H], BF16, tag="po", name="po")
        wo_sb = finwp.tile([128, IC * H], BF16, tag="wo", name="wo")
        for k in range(KC):
            nc.sync.dma_start(uo_sb[:, k * H: (k + 1) * H], uo_in[k, :, :])
            nc.sync.dma_start(po_sb[:, k * H: (k + 1) * H], po_in[k, :, :])
        for c in range(IC):
            nc.sync.dma_start(wo_sb[:, c * H: (c + 1) * H], wo_in[c, :, :])
        bo_sb = finp.tile([1, H], BF16, tag="bo", name="bo")
        nc.sync.dma_start(bo_sb[:], biaso_in[:])
        gidx = finp.tile([16, 8], I32, tag="gidx", name="gidx")
        nc.sync.dma_start(gidx[:], gidx_in[:])
        xf_sb = finp.tile([NB, I_DIM], BF16, tag="xf", name="xf")
        nc.sync.dma_start(xf_sb[:], xf_in[:])

        # gather c rows: c_hist row L*128+32g+r holds (row r, h[256g:256g+256))
        HG = 256
        cout_b = finp.tile([NB, H], F32, tag="cout", name="cout")
        cin_b = finp.tile([NB, H], F32, tag="cin", name="cin")
        for g in range(NS):
            nc.gpsimd.indirect_dma_start(
                out=cout_b[:, HG * g: HG * (g + 1)], out_offset=None,
                in_=c_hist[:],
                in_offset=bass.IndirectOffsetOnAxis(ap=gidx[:, g: g + 1], axis=0),
            )
            nc.gpsimd.indirect_dma_start(
                out=cin_b[:, HG * g: HG * (g + 1)], out_offset=None,
                in_=c_hist[:],
                in_offset=bass.IndirectOffsetOnAxis(ap=gidx[:, 4 + g: 5 + g], axis=0),
            )

        def transpose_to_bf16(src_b, nm, nchunk, ident, psum_dtype):
            dst = finp.tile([128, 16 * nchunk], BF16, tag=nm, name=nm)
            pstf = pstfp.tile([128, 128], psum_dtype, tag="pstf" + nm,
                              name="pstf" + nm)
            for k in range(nchunk):
                nc.tensor.transpose(
                    pstf[:, 16 * k: 16 * k + 16],
                    src_b[0:NB, 128 * k: 128 * k + 128],
                    ident[0:16, 0:16],
                )
            nc.vector.tensor_copy(dst[:], pstf[:, 0: 16 * nchunk])
            return dst

        xtT = transpose_to_bf16(xf_sb, "xtT", IC, id16, BF16)
        cinT = transpose_to_bf16(cin_b, "cinT", KC, id16f, F32)
        coutT = transpose_to_bf16(cout_b, "coutT", KC, id16f, F32)

        ps_o = psfp.tile([NB, H], F32)
        ps_po = psf2p.tile([NB, H], F32)
        for half in range(2):
            cs = slice(512 * half, 512 * half + 512)
            nc.tensor.matmul(ps_o[:, cs], ones1[0:1, 0:NB], bo_sb[0:1, cs],
                             start=True, stop=False)
            for c in range(IC):
                nc.tensor.matmul(
                    ps_o[:, cs], xtT[:, 16 * c: 16 * c + 16],
                    wo_sb[:, c * H + 512 * half: c * H + 512 * half + 512],
                    start=False, stop=False)
            for k in range(KC):
                nc.tensor.matmul(
                    ps_o[:, cs], cinT[:, 16 * k: 16 * k + 16],
                    uo_sb[:, k * H + 512 * half: k * H + 512 * half + 512],
                    start=False, stop=False)
                nc.tensor.matmul(
                    ps_po[:, cs], coutT[:, 16 * k: 16 * k + 16],
                    po_sb[:, k * H + 512 * half: k * H + 512 * half + 512],
                    start=(k == 0), stop=(k == KC - 1))
        tpo = finp.tile([NB, H], BF16, tag="tpo", name="tpo")
        nc.scalar.activation(tpo[:], ps_po[:], AF.Tanh)
        for half in range(2):
            cs = slice(512 * half, 512 * half + 512)
            nc.tensor.matmul(ps_o[:, cs], id16[0:NB, :], tpo[:, cs],
                             start=False, stop=True, skip_group_check=True)
        o_sb = finp.tile([NB, H], F32, tag="osb", name="osb")
        nc.scalar.activation(o_sb[:], ps_o[:], AF.Sigmoid)
        tanc = finp.tile([NB, H], F32, tag="tanc", name="tanc")
        nc.scalar.activation(tanc[:], cout_b[:], AF.Tanh)
        h_sb = finp.tile([NB, H], F32, tag="hsb", name="hsb")
        nc.vector.tensor_mul(h_sb[:], o_sb[:], tanc[:])
        nc.sync.dma_start(h_out[:], h_sb[:])


def build_program(t_steps=T_FULL, parts=("pre", "loop", "fin"),
                  c_hist_out=False):
    t = t_steps
    assert (t * NB) % 128 == 0

    nc = bacc.Bacc(None, target_bir_lowering=False, debug=False)
    dp = nc.declare_dram_parameter
    up_in = dp("up", [KC, 128, UPW], BF16, isOutput=False)
    w_in = dp("w", [IC, 128, NS * 768], BF16, isOutput=False)
    bias_bc_in = dp("bias_bc", [128, NS * 768], BF16, isOutput=False)
    uo_in = dp("uo", [KC, 128, H], BF16, isOutput=False)
    po_in = dp("po", [KC, 128, H], BF16, isOutput=False)
    wo_in = dp("wo", [IC, 128, H], BF16, isOutput=False)
    biaso_in = dp("biaso", [1, H], BF16, isOutput=False)
    xT_in = dp("xT", [IC, 128, t * NB], BF16, isOutput=False)
    xf_in = dp("x_f", [NB, I_DIM], BF16, isOutput=False)
    gidx_in = dp("gidx", [16, 8], I32, isOutput=False)
    id16_in = dp("id16", [128, 16], BF16, isOutput=False)
    id16f_in = dp("id16f", [128, 16], F32, isOutput=False)
    id128_in = dp("id128", [128, 128], F32, isOutput=False)
    id128b_in = dp("id128b", [128, 128], BF16, isOutput=False)
    ones1_in = dp("ones1", [1, 128], BF16, isOutput=False)
    h_out = dp("h_out", [NB, H], F32, isOutput=True)

    xw_hist = nc.dram_tensor("xw_hist", [t * 128, XWW], BF16)
    if c_hist_out:
        c_hist = dp("c_hist", [(t + 1) * 128, 256], F32, isOutput=True)
    else:
        c_hist = nc.dram_tensor("c_hist", [(t + 1) * 128, 256], F32)

    with tile.TileContext(nc) as tc:
        with tc.tile_pool(name="const", bufs=1) as constp:
            id16 = constp.tile([128, 16], BF16)
            nc.sync.dma_start(id16[:], id16_in[:])
            id16f = constp.tile([128, 16], F32)
            nc.sync.dma_start(id16f[:], id16f_in[:])
            ones1 = constp.tile([1, 128], BF16)
            nc.sync.dma_start(ones1[:], ones1_in[:])
            id128 = constp.tile([128, 128], F32)
            nc.sync.dma_start(id128[:], id128_in[:])
            id128b = constp.tile([128, 128], BF16)
            nc.sync.dma_start(id128b[:], id128b_in[:])
            cst = dict(id16=id16, id16f=id16f, ones1=ones1, id128=id128,
                       id128b=id128b)

            if "pre" in parts:
                _phase_precompute(nc, tc, t, xT_in, w_in, bias_bc_in, xw_hist)
            if "loop" in parts:
                _phase_loop(nc, tc, cst, t, up_in, xw_hist, c_hist)
            if "fin" in parts:
                _phase_finalize(nc, tc, cst, uo_in, po_in, wo_in, biaso_in,
                                gidx_in, xf_in, c_hist, h_out)
            else:
                with tc.tile_pool(name="dummy", bufs=1) as dummyp:
                    hz = dummyp.tile([NB, H], F32)
                    nc.vector.memset(hz[:], 0.0)
                    nc.sync.dma_start(h_out[:], hz[:])

    nc.compile()
    return nc


# ------------------------------------------------------- full host-side kernel

def make_in_maps(inputs, t_steps):
    x = np.asarray(inputs["x"], np.float32)
    lens = np.asarray(inputs["lens"]).astype(np.int64)
    wp = pack_weights(np.asarray(inputs["U"], np.float32),
                      np.asarray(inputs["P"], np.float32),
                      np.asarray(inputs["W"], np.float32),
                      np.asarray(inputs["P_o"], np.float32),
                      np.asarray(inputs["bias"], np.float32))
    cp = pack_consts()
    shared = {**wp, **cp}
    in_maps = []
    for core in range(8):
        sl = slice(core * NB, (core + 1) * NB)
        ci = pack_core_inputs(x[sl], lens[sl], t_steps)
        in_maps.append({**shared, **ci})
    return in_maps


def run(inputs, t_steps=T_FULL, trace=False, parts=("pre", "loop", "fin"),
        c_hist_out=False, nc_cache={}):
    from concourse.bass_utils import run_bass_kernel_spmd

    in_maps = make_in_maps(inputs, t_steps)
    key = (t_steps, parts, c_hist_out)
    if key not in nc_cache:
        nc_cache[key] = build_program(t_steps, parts, c_hist_out)
    nc = nc_cache[key]

    res = run_bass_kernel_spmd(nc, in_maps, list(range(8)), trace=trace)
    h = np.concatenate([res.results[i]["h_out"] for i in range(8)], axis=0)
    return h.astype(np.float32), res


# ======================================================================
# Public entry point: full inputs in, full output out.
# ======================================================================

LAST_EXEC_NS = None


def kernel(**inputs):
    """Peephole-LSTM forward; returns h at t=lens-1 for each row: [B, H] f32."""
    global LAST_EXEC_NS
    import os
    trace = bool(os.environ.get("BASS_TRACE"))
    h, res = run(inputs, t_steps=T_FULL, trace=trace)
    if res.exec_time_ns is not None:
        LAST_EXEC_NS = res.exec_time_ns
    return h
